# revision 10
# baseline (speedup 1.0000x reference)
"""Clustered attention Trainium2 kernel (8-core SPMD, sharded along v).

Math (per batch b):
    sum_tot = key.sum(axis=2)                          # (L, D)
    S[i,k,j] = query[i,k,:] . sum_tot[j,:]
    A = softmax_j(scale * S  masked to label[i]==label[j])
    out[i,k,:] = sum_j A[i,k,j] * value[j,k,:]

Device layout (per (b, v) pair, v sharded 8 ways -> Vc=8 per core):
    S^T tiles (j on partitions, i free) = (sum_tot^T slice).T @ q^T
    no-max softmax: A' = exp(scale*S) * mask01   (scores bounded ~|44|, safe in fp32/bf16 exponent range)
    value packed with a trailing ones column -> the A'^T @ [V|1] matmul yields
    both the numerator and the softmax denominator in one accumulation group.
    Normalize with a per-partition reciprocal multiply.

Precision: bf16 matmuls with a 3-term hi/lo split of the scores product
    S ~= q_hi.s_hi + q_hi.s_lo + q_lo.s_hi   (end-to-end rel err ~1.7e-3)
"""

import numpy as np
import ml_dtypes

import concourse.bass as bass
import concourse.tile as tile
from concourse import mybir
from concourse.bass import ts
from concourse.bass_utils import run_bass_kernel_spmd

BF16 = ml_dtypes.bfloat16
F32 = np.float32

# Problem shape (hardcoded per contract: kernel.py is self-contained).
B, L, V, D = 2, 512, 64, 128
N_CORES = 8
VC = V // N_CORES          # v slots per core
T = L // 128               # 128-row tiles along L
SCALE = 1.0 / float(np.sqrt(D))
SPLIT3 = True              # 3-term hi/lo split for the scores matmul


# walrus's sync-wait lowering only tolerates 1 wait on DMA instructions and
# 2 on compute instructions; Tile can emit more. Hoist the excess onto
# preceding same-engine NoOps (the engine sequencer performs waits in order,
# so semantics are unchanged).
_WAIT_EXEMPT = {
    "InstEventSemaphore", "InstNoOp", "InstCall", "InstISA",
    "InstUnconditionalBranch", "InstCompareAndBranch", "InstRegisterMove",
    "InstBranchHint", "InstHalt",
}


def _split_waits(nc, dma_cap=1, compute_cap=1):
    fn = nc.m.functions[0]
    for blk in fn.blocks:
        il = blk.instructions
        new = []
        changed = False
        for inst in il:
            tname = type(inst).__name__
            si = inst.sync_info
            if si is not None and tname not in _WAIT_EXEMPT:
                cap = dma_cap if tname in ("InstDMACopy", "InstDMA") else compute_cap
                waits = list(si.on_wait)
                if len(waits) > cap:
                    excess, keep = waits[:-cap], waits[-cap:]
                    for w in excess:
                        nop = mybir.InstNoOp(
                            name=nc.get_next_instruction_name(),
                            sync_info=mybir.SyncInfo(on_wait=[w], on_update=[]),
                            engine=inst.engine,
                            bass_nofuse=True,
                        )
                        new.append(nop)
                    inst.sync_info = mybir.SyncInfo(
                        on_wait=keep, on_update=list(si.on_update)
                    )
                    changed = True
            new.append(inst)
        if changed:
            blk.instructions = new


def _build_bass():
    nc = bass.Bass()
    bf = mybir.dt.bfloat16
    f32 = mybir.dt.float32

    qhi = nc.dram_tensor("qhi", (B, VC, D, L), bf, kind="ExternalInput")
    qlo = nc.dram_tensor("qlo", (B, VC, D, L), bf, kind="ExternalInput")
    shi = nc.dram_tensor("shi", (B, D, L), bf, kind="ExternalInput")
    slo = nc.dram_tensor("slo", (B, D, L), bf, kind="ExternalInput")
    vp = nc.dram_tensor("vp", (B, VC, 128, T, D + 1), bf, kind="ExternalInput")
    mk = nc.dram_tensor("mk", (B, 128, T, L), bf, kind="ExternalInput")
    out = nc.dram_tensor("out", (B, L, VC, D), f32, kind="ExternalOutput")

    with tile.TileContext(nc) as tc:
        with (
            tc.tile_pool(name="consts", bufs=1) as cpool,
            tc.tile_pool(name="qin", bufs=6) as qpool,
            tc.tile_pool(name="vin", bufs=6) as vpool,
            tc.tile_pool(name="aw", bufs=2) as apool,
            tc.tile_pool(name="og", bufs=2) as opool,
            tc.tile_pool(name="rc", bufs=8) as rpool,
            tc.tile_pool(name="spsum", bufs=2, space="PSUM") as spsum,
            tc.tile_pool(name="opsum", bufs=4, space="PSUM") as opsum,
        ):
            sh_all = cpool.tile([128, B, L], bf)
            sl_all = cpool.tile([128, B, L], bf)
            mk_all = cpool.tile([128, B, T, L], bf)
            nc.sync.dma_start(out=sh_all, in_=shi[:, :, :].rearrange("b d l -> d b l"))
            nc.sync.dma_start(out=sl_all, in_=slo[:, :, :].rearrange("b d l -> d b l"))
            nc.sync.dma_start(out=mk_all, in_=mk[:, :, :, :].rearrange("b p t l -> p b t l"))
            for b in range(B):
                shb = sh_all[:, b, :]
                slb = sl_all[:, b, :]
                mkb = mk_all[:, b, :, :]
                for v in range(VC):
                    qh = qpool.tile([128, L], bf)
                    nc.sync.dma_start(out=qh, in_=qhi[b, v])
                    if SPLIT3:
                        ql = qpool.tile([128, L], bf)
                        nc.sync.dma_start(out=ql, in_=qlo[b, v])
                    vv = vpool.tile([128, T, D + 1], bf)
                    nc.sync.dma_start(out=vv, in_=vp[b, v])

                    # A'^T, all four j-tiles: partitions = j%128, free = (jt, i)
                    at = apool.tile([128, T, L], bf)
                    for g in range(T // 2):
                        ps = spsum.tile([128, 2, L], mybir.dt.float32)
                        for h in range(2):
                            jt = 2 * g + h
                            nc.tensor.matmul(
                                ps[:, h, :], shb[:, ts(jt, 128)], qh,
                                start=True, stop=not SPLIT3,
                            )
                            if SPLIT3:
                                nc.tensor.matmul(
                                    ps[:, h, :], slb[:, ts(jt, 128)], qh,
                                    start=False, stop=False,
                                )
                                nc.tensor.matmul(
                                    ps[:, h, :], shb[:, ts(jt, 128)], ql,
                                    start=False, stop=True,
                                )
                        nc.scalar.activation(
                            at[:, 2 * g:2 * g + 2, :], ps,
                            mybir.ActivationFunctionType.Exp, scale=SCALE,
                        )
                        nc.vector.tensor_mul(
                            at[:, 2 * g:2 * g + 2, :],
                            at[:, 2 * g:2 * g + 2, :],
                            mkb[:, 2 * g:2 * g + 2, :],
                        )

                    og = opool.tile([128, T, D], mybir.dt.float32)
                    for it in range(T):
                        ops = opsum.tile([128, D + 1], mybir.dt.float32)
                        for jt in range(T):
                            nc.tensor.matmul(
                                ops, at[:, jt, ts(it, 128)], vv[:, jt, :],
                                start=(jt == 0), stop=(jt == T - 1),
                            )
                        rc = rpool.tile([128, 1], mybir.dt.float32)
                        nc.vector.reciprocal(rc, ops[:, D:D + 1])
                        nc.vector.tensor_scalar_mul(og[:, it, :], ops[:, 0:D], rc)
                    nc.sync.dma_start(
                        out=out[b, :, v, :].rearrange("(t p) d -> p t d", p=128),
                        in_=og,
                    )
    _split_waits(nc)
    return nc


_BASS_CACHE = None


def _get_bass():
    global _BASS_CACHE
    if _BASS_CACHE is None:
        _BASS_CACHE = _build_bass()
    return _BASS_CACHE


def _prepare_inputs(query, key, value, label_arr):
    """Host-side packing: transposes/casts/hi-lo splits + per-core sharding."""
    query = np.asarray(query, dtype=F32)
    key = np.asarray(key, dtype=F32)
    value = np.asarray(value, dtype=F32)
    lab = np.asarray(label_arr)

    sum_tot = key.sum(axis=2)                      # (B, L, D) f32
    sT = np.ascontiguousarray(sum_tot.transpose(0, 2, 1))     # (B, D, L)
    s_hi = sT.astype(BF16)
    s_lo = (sT - s_hi.astype(F32)).astype(BF16)

    qT = np.ascontiguousarray(query.transpose(0, 2, 3, 1))    # (B, V, D, L)
    q_hi = qT.astype(BF16)
    q_lo = (qT - q_hi.astype(F32)).astype(BF16)

    # value packed (B, V, 128, T, D+1) with ones in the last column
    v4 = value.reshape(B, T, 128, V, D).transpose(0, 3, 2, 1, 4)  # (B,V,128,T,D)
    vp = np.empty((B, V, 128, T, D + 1), dtype=BF16)
    vp[..., :D] = v4.astype(BF16)
    vp[..., D] = np.ones((), dtype=BF16)

    # mask (B, 128, T, L): mask[b, jm, t, i] = lab[b, t*128+jm] == lab[b, i]
    labr = lab.reshape(B, T, 128)
    m = (labr[:, :, :, None] == lab[:, None, None, :])        # (B, T, 128, L)
    mk = np.ascontiguousarray(m.transpose(0, 2, 1, 3)).astype(BF16)

    in_maps = []
    for c in range(N_CORES):
        sl = slice(c * VC, (c + 1) * VC)
        in_maps.append({
            "qhi": np.ascontiguousarray(q_hi[:, sl]),
            "qlo": np.ascontiguousarray(q_lo[:, sl]),
            "shi": s_hi,
            "slo": s_lo,
            "vp": np.ascontiguousarray(vp[:, sl]),
            "mk": mk,
        })
    return in_maps


def kernel(query, key, value, label_arr):
    nc = _get_bass()
    in_maps = _prepare_inputs(query, key, value, label_arr)
    res = run_bass_kernel_spmd(nc, in_maps, core_ids=list(range(N_CORES)))
    full = np.empty((B, L, V, D), dtype=F32)
    for c in range(N_CORES):
        full[:, :, c * VC:(c + 1) * VC, :] = res.results[c]["out"]
    return full


# revision 11
# speedup vs baseline: 1.2769x; 1.2769x over previous
"""Clustered attention Trainium2 kernel (8-core SPMD, sharded along v).

Math (per batch b):
    sum_tot = key.sum(axis=2)                          # (L, D)
    S[i,k,j] = query[i,k,:] . sum_tot[j,:]
    A = softmax_j(scale * S  masked to label[i]==label[j])
    out[i,k,:] = sum_j A[i,k,j] * value[j,k,:]

Device layout (per (b, v) pair, v sharded 8 ways -> Vc=8 per core):
    S^T tiles (j on partitions, i free) = (sum_tot^T slice).T @ q^T,
    computed in float32r (full-rate fp32 matmul at N>=256).
    no-max softmax: A' = exp(scale*S) * mask01   (scores bounded ~|44|, safe in
    fp32/bf16 exponent range).
    value packed bf16 with a trailing ones column -> the A'^T @ [V|1] matmul
    yields both the numerator and the softmax denominator in one accumulation
    group. Normalize with a per-partition reciprocal multiply.
"""

import numpy as np
import ml_dtypes

import concourse.bass as bass
import concourse.tile as tile
from concourse import mybir
from concourse.bass import ts
from concourse.bass_utils import run_bass_kernel_spmd

BF16 = ml_dtypes.bfloat16
F32 = np.float32

# Problem shape (hardcoded per contract: kernel.py is self-contained).
B, L, V, D = 2, 512, 64, 128
N_CORES = 8
VC = V // N_CORES          # v slots per core
T = L // 128               # 128-row tiles along L
SCALE = 1.0 / float(np.sqrt(D))


# walrus's sync-wait lowering only tolerates 1 wait per instruction; Tile can
# emit more. Hoist the excess onto preceding same-engine NoOps (the engine
# sequencer performs waits in order, so semantics are unchanged).
_WAIT_EXEMPT = {
    "InstEventSemaphore", "InstNoOp", "InstCall", "InstISA",
    "InstUnconditionalBranch", "InstCompareAndBranch", "InstRegisterMove",
    "InstBranchHint", "InstHalt",
}


def _split_waits(nc, dma_cap=1, compute_cap=1):
    fn = nc.m.functions[0]
    for blk in fn.blocks:
        il = blk.instructions
        new = []
        changed = False
        for inst in il:
            tname = type(inst).__name__
            si = inst.sync_info
            if si is not None and tname not in _WAIT_EXEMPT:
                cap = dma_cap if tname in ("InstDMACopy", "InstDMA") else compute_cap
                waits = list(si.on_wait)
                if len(waits) > cap:
                    excess, keep = waits[:-cap], waits[-cap:]
                    for w in excess:
                        nop = mybir.InstNoOp(
                            name=nc.get_next_instruction_name(),
                            sync_info=mybir.SyncInfo(on_wait=[w], on_update=[]),
                            engine=inst.engine,
                            bass_nofuse=True,
                        )
                        new.append(nop)
                    inst.sync_info = mybir.SyncInfo(
                        on_wait=keep, on_update=list(si.on_update)
                    )
                    changed = True
            new.append(inst)
        if changed:
            blk.instructions = new


def _build_bass():
    nc = bass.Bass()
    bf = mybir.dt.bfloat16
    f32 = mybir.dt.float32
    f32r = mybir.dt.float32r

    qf = nc.dram_tensor("qf", (B, VC, D, L), f32r, kind="ExternalInput")
    sf = nc.dram_tensor("sf", (B, D, L), f32r, kind="ExternalInput")
    vp = nc.dram_tensor("vp", (B, VC, 128, T, D + 1), bf, kind="ExternalInput")
    mk = nc.dram_tensor("mk", (B, 128, T, L), bf, kind="ExternalInput")
    out = nc.dram_tensor("out", (B, L, VC, D), f32, kind="ExternalOutput")

    with tile.TileContext(nc) as tc:
        with (
            tc.tile_pool(name="consts", bufs=1) as cpool,
            tc.tile_pool(name="qin", bufs=6) as qpool,
            tc.tile_pool(name="vin", bufs=6) as vpool,
            tc.tile_pool(name="aw", bufs=2) as apool,
            tc.tile_pool(name="og", bufs=2) as opool,
            tc.tile_pool(name="rc", bufs=8) as rpool,
            tc.tile_pool(name="spsum", bufs=2, space="PSUM") as spsum,
            tc.tile_pool(name="opsum", bufs=4, space="PSUM") as opsum,
        ):
            sf_all = cpool.tile([128, B, L], f32r)
            mk_all = cpool.tile([128, B, T, L], bf)
            nc.sync.dma_start(out=sf_all, in_=sf[:, :, :].rearrange("b d l -> d b l"))
            nc.sync.dma_start(out=mk_all, in_=mk[:, :, :, :].rearrange("b p t l -> p b t l"))
            for b in range(B):
                sfb = sf_all[:, b, :]
                mkb = mk_all[:, b, :, :]
                for v in range(VC):
                    qt = qpool.tile([128, L], f32r)
                    nc.sync.dma_start(out=qt, in_=qf[b, v])
                    vv = vpool.tile([128, T, D + 1], bf)
                    nc.sync.dma_start(out=vv, in_=vp[b, v])

                    # A'^T, all four j-tiles: partitions = j%128, free = (jt, i)
                    at = apool.tile([128, T, L], bf)
                    for g in range(T // 2):
                        ps = spsum.tile([128, 2, L], f32)
                        for h in range(2):
                            jt = 2 * g + h
                            nc.tensor.matmul(
                                ps[:, h, :], sfb[:, ts(jt, 128)], qt,
                                start=True, stop=True,
                            )
                        nc.scalar.activation(
                            at[:, 2 * g:2 * g + 2, :], ps,
                            mybir.ActivationFunctionType.Exp, scale=SCALE,
                        )
                        nc.vector.tensor_mul(
                            at[:, 2 * g:2 * g + 2, :],
                            at[:, 2 * g:2 * g + 2, :],
                            mkb[:, 2 * g:2 * g + 2, :],
                        )

                    og = opool.tile([128, T, D], f32)
                    for it in range(T):
                        ops = opsum.tile([128, D + 1], f32)
                        for jt in range(T):
                            nc.tensor.matmul(
                                ops, at[:, jt, ts(it, 128)], vv[:, jt, :],
                                start=(jt == 0), stop=(jt == T - 1),
                            )
                        rc = rpool.tile([128, 1], f32)
                        nc.vector.reciprocal(rc, ops[:, D:D + 1])
                        nc.vector.tensor_scalar_mul(og[:, it, :], ops[:, 0:D], rc)
                    nc.sync.dma_start(
                        out=out[b, :, v, :].rearrange("(t p) d -> p t d", p=128),
                        in_=og,
                    )
    _split_waits(nc)
    return nc


_BASS_CACHE = None


def _get_bass():
    global _BASS_CACHE
    if _BASS_CACHE is None:
        _BASS_CACHE = _build_bass()
    return _BASS_CACHE


def _prepare_inputs(query, key, value, label_arr):
    """Host-side packing: transposes/casts + per-core sharding."""
    query = np.asarray(query, dtype=F32)
    key = np.asarray(key, dtype=F32)
    value = np.asarray(value, dtype=F32)
    lab = np.asarray(label_arr)

    sum_tot = key.sum(axis=2)                                 # (B, L, D) f32
    sT = np.ascontiguousarray(sum_tot.transpose(0, 2, 1))     # (B, D, L)
    qT = np.ascontiguousarray(query.transpose(0, 2, 3, 1))    # (B, V, D, L)

    # value packed (B, V, 128, T, D+1) with ones in the last column
    v4 = value.reshape(B, T, 128, V, D).transpose(0, 3, 2, 1, 4)  # (B,V,128,T,D)
    vp = np.empty((B, V, 128, T, D + 1), dtype=BF16)
    vp[..., :D] = v4.astype(BF16)
    vp[..., D] = np.ones((), dtype=BF16)

    # mask (B, 128, T, L): mask[b, jm, t, i] = lab[b, t*128+jm] == lab[b, i]
    labr = lab.reshape(B, T, 128)
    m = (labr[:, :, :, None] == lab[:, None, None, :])        # (B, T, 128, L)
    mk = np.ascontiguousarray(m.transpose(0, 2, 1, 3)).astype(BF16)

    in_maps = []
    for c in range(N_CORES):
        sl = slice(c * VC, (c + 1) * VC)
        in_maps.append({
            "qf": np.ascontiguousarray(qT[:, sl]),
            "sf": sT,
            "vp": np.ascontiguousarray(vp[:, sl]),
            "mk": mk,
        })
    return in_maps


def kernel(query, key, value, label_arr):
    nc = _get_bass()
    in_maps = _prepare_inputs(query, key, value, label_arr)
    res = run_bass_kernel_spmd(nc, in_maps, core_ids=list(range(N_CORES)))
    full = np.empty((B, L, V, D), dtype=F32)
    for c in range(N_CORES):
        full[:, :, c * VC:(c + 1) * VC, :] = res.results[c]["out"]
    return full
